# revision 18
# baseline (speedup 1.0000x reference)
"""Trainium2 Bass kernel for the ActorCritic GNN (ChebConv + perm-shuffle + policy/value GEMV).

Strategy (8 NeuronCores, SPMD single NEFF):
  - Row-shard the 16N=65536 reduction dim of pol_W/val_W: core c owns 8192 rows.
  - Edge-shard the ChebConv message passing (8192 edges/core, CSR dst-sorted by
    the host as part of the sharding format). Segment sums are computed
    scatter-free: gather edge values in dst-sorted order, prefix-scan them on
    the vector engine (per-partition scan + triangular-matmul cross-partition
    offsets), write the prefix table to HBM, then two row-gathers at the CSR
    row pointers and a subtract give each node's aggregate. Node aggregates
    are AllReduced across cores.
  - The 1 GiB pol_W stream is the roofline; per-core k-rows are host-sorted so
    slots whose perm entry hits the virtual-node half (which depends only on
    the tiny vnr GEMM) come first: their matmuls start early and overlap the
    whole graph front-end.
  - Softmax + value epilogue replicated on all cores after a final AllReduce
    of the logit/value partials.
"""

import numpy as np

N = 4096          # nodes
E = 65536         # edges
NCORES = 8
EPC = E // NCORES          # 8192 edges per core
KPC = 2 * 8 * N // NCORES  # 8192 k-rows per core
KT = KPC // 128            # 64 k-tiles per core
EL = 64                    # indirect-DMA element size in fp32 (256 B slots)
IDX_CHUNK = 1024           # indices per dma_gather (single_packet=True limit)

_BUILD_CACHE = {}


def _wrap16(idx):
    """SWDGE index layout: [128, n/16] int16, idx i at partition i%16 col
    i//16, replicated across the 8 gpsimd cores (16-partition groups)."""
    n = idx.shape[0]
    a = np.empty((128, n // 16), np.int16)
    w = idx.reshape(n // 16, 16).T
    for g in range(8):
        a[g * 16:(g + 1) * 16, :] = w
    return a


def _build(vt):
    import concourse.bacc as bacc
    import concourse.mybir as mybir
    import concourse.tile as tile
    from concourse.masks import make_identity

    f32 = mybir.dt.float32
    f32r = mybir.dt.float32r
    i16 = mybir.dt.int16
    Alu = mybir.AluOpType
    Act = mybir.ActivationFunctionType
    X = mybir.AxisListType.X

    ct = KT - vt  # conv-gathered k-tiles (stage B)

    nc = bacc.Bacc(None, target_bir_lowering=False)

    # ---- inputs (per core) ----
    pol_w = nc.dram_tensor("pol_w", [KPC, N], f32r, kind="ExternalInput")
    val_w = nc.dram_tensor("val_w", [128, KT], f32, kind="ExternalInput")
    pol_b = nc.dram_tensor("pol_b", [N], f32, kind="ExternalInput")
    val_b = nc.dram_tensor("val_b", [1], f32, kind="ExternalInput")
    x_in = nc.dram_tensor("x_in", [N, 4], f32, kind="ExternalInput")
    w0t = nc.dram_tensor("w0t", [1, 32], f32, kind="ExternalInput")
    w1t = nc.dram_tensor("w1t", [1, 32], f32, kind="ExternalInput")
    w2t = nc.dram_tensor("w2t", [1, 32], f32, kind="ExternalInput")
    chb = nc.dram_tensor("chb", [1, 8], f32, kind="ExternalInput")
    vnx = nc.dram_tensor("vnx", [1, 2], f32, kind="ExternalInput")
    vnw = nc.dram_tensor("vnw", [1, 16], f32, kind="ExternalInput")
    vnb = nc.dram_tensor("vnb", [1, 8], f32, kind="ExternalInput")
    rp_src = nc.dram_tensor("rp_src", [N + 1], f32, kind="ExternalInput")
    srcg_i = nc.dram_tensor("srcg_i", [128, EPC // 16], i16, kind="ExternalInput")
    hi_i = nc.dram_tensor("hi_i", [128, N // 16], i16, kind="ExternalInput")
    lo_i = nc.dram_tensor("lo_i", [128, N // 16], i16, kind="ExternalInput")
    if vt > 0:
        oh8 = nc.dram_tensor("oh8", [128, vt, 8], f32, kind="ExternalInput")
    if ct > 0:
        grp_i = nc.dram_tensor("grp_i", [128, ct * 8], i16, kind="ExternalInput")
        oh64 = nc.dram_tensor("oh64", [128, ct, 64], f32, kind="ExternalInput")

    probs_o = nc.dram_tensor("probs", [N], f32, kind="ExternalOutput")
    value_o = nc.dram_tensor("value", [1], f32, kind="ExternalOutput")

    RG = [list(range(NCORES))]
    GCH = 2048  # gather chunk (single_packet=False)

    with tile.TileContext(nc) as tc:
        with (
            tc.tile_pool(name="sb", bufs=1) as sb,
            tc.tile_pool(name="stream", bufs=4) as stream,
            tc.tile_pool(name="ps", bufs=1, space="PSUM") as ps,
            tc.tile_pool(name="dram", bufs=1, space="DRAM") as dram,
        ):
            # ---- internal DRAM ----
            xd_tab = dram.tile([N, EL], f32)
            zd_tab = dram.tile([N, EL], f32)
            p1_tab = dram.tile([EPC + 1, EL], f32)
            p2_tab = dram.tile([EPC + 1, EL], f32)
            y_cmp = dram.tile([N, 4], f32)
            y_ar = dram.tile([N, 4], f32)
            lz_cmp = dram.tile([N, 8], f32)
            lz_ar = dram.tile([N, 8], f32)
            f_tab = dram.tile([1024, EL], f32)
            vg_stage = dram.tile([1, 8], f32)
            lg_in = dram.tile([1, 4104], f32)
            lg_out = dram.tile([1, 4104], f32)

            # ---- index / constant loads ----
            srcg_t = sb.tile([128, EPC // 16], i16)
            hi_t = sb.tile([128, N // 16], i16)
            lo_t = sb.tile([128, N // 16], i16)
            nc.scalar.dma_start(out=srcg_t[:], in_=srcg_i[:])
            nc.scalar.dma_start(out=hi_t[:], in_=hi_i[:])
            nc.scalar.dma_start(out=lo_t[:], in_=lo_i[:])
            if ct > 0:
                grp_t = sb.tile([128, ct * 8], i16)
                nc.scalar.dma_start(out=grp_t[:], in_=grp_i[:])
                oh64_t = sb.tile([128, ct, 64], f32)
                nc.scalar.dma_start(out=oh64_t[:], in_=oh64[:])
            x_t = sb.tile([128, 32, 4], f32)
            nc.scalar.dma_start(out=x_t[:], in_=x_in[:].rearrange("(p c) f -> p c f", p=128))

            # zero row 0 of the prefix tables
            ztile = sb.tile([1, EL], f32)
            nc.vector.memset(ztile[:], 0.0)
            nc.scalar.dma_start(out=p1_tab[0:1, :], in_=ztile[:])
            nc.scalar.dma_start(out=p2_tab[0:1, :], in_=ztile[:])

            # ---- degree from CSR row pointers: deg = rp_src[n+1] - rp_src[n] ----
            rp_hi = sb.tile([128, 32], f32)
            rp_lo = sb.tile([128, 32], f32)
            nc.scalar.dma_start(out=rp_hi[:], in_=rp_src[1:N + 1].rearrange("(p c) -> p c", p=128))
            nc.scalar.dma_start(out=rp_lo[:], in_=rp_src[0:N].rearrange("(p c) -> p c", p=128))
            degc = sb.tile([128, 32], f32)
            nc.vector.tensor_sub(out=degc[:], in0=rp_hi[:], in1=rp_lo[:])
            d1 = sb.tile([128, 32], f32)
            nc.vector.tensor_scalar_max(out=d1[:], in0=degc[:], scalar1=1.0)
            dsq = sb.tile([128, 32], f32)
            nc.scalar.activation(out=dsq[:], in_=d1[:], func=Act.Sqrt)
            drec = sb.tile([128, 32], f32)
            nc.vector.reciprocal(out=drec[:], in_=dsq[:])
            dmask = sb.tile([128, 32], f32)
            nc.vector.tensor_single_scalar(out=dmask[:], in_=degc[:], scalar=0.0, op=Alu.is_gt)
            dinv = sb.tile([128, 32], f32)
            nc.vector.tensor_mul(out=dinv[:], in0=drec[:], in1=dmask[:])

            # ---- cheb weight prep (broadcast to all partitions) ----
            w0_t = sb.tile([128, 32], f32)
            w1_t = sb.tile([128, 32], f32)
            w2_t = sb.tile([128, 32], f32)
            nc.gpsimd.dma_start(out=w0_t[:], in_=w0t[:].to_broadcast([128, 32]))
            nc.gpsimd.dma_start(out=w1_t[:], in_=w1t[:].to_broadcast([128, 32]))
            nc.gpsimd.dma_start(out=w2_t[:], in_=w2t[:].to_broadcast([128, 32]))
            chb_t = sb.tile([128, 8], f32)
            nc.gpsimd.dma_start(out=chb_t[:], in_=chb[:].to_broadcast([128, 8]))
            w02_t = sb.tile([128, 32], f32)
            nc.vector.tensor_sub(out=w02_t[:], in0=w0_t[:], in1=w2_t[:])
            wcat_t = sb.tile([128, 8, 8], f32)
            nc.vector.tensor_copy(out=wcat_t[:, :, 0:4], in_=w1_t[:].rearrange("p (j g) -> p j g", g=4))
            nc.vector.tensor_scalar_mul(out=wcat_t[:, :, 4:8], in0=w2_t[:].rearrange("p (j g) -> p j g", g=4), scalar1=2.0)

            # xw02 = x @ (W0 - W2)  [128, 32, 8]
            xw02_p = sb.tile([128, 32, 8, 4], f32)
            nc.vector.tensor_mul(
                out=xw02_p[:],
                in0=x_t[:][:, :, None, :].to_broadcast([128, 32, 8, 4]),
                in1=w02_t[:].rearrange("p (j g) -> p j g", g=4)[:, None, :, :].to_broadcast([128, 32, 8, 4]),
            )
            xw02 = sb.tile([128, 32, 8], f32)
            nc.vector.tensor_reduce(out=xw02[:][:, :, :, None], in_=xw02_p[:], axis=X, op=Alu.add)


            # ---- virtual-node row: vg = vnr_x[j] @ vnr_W + vnr_b  [1, 8] ----
            vnx_t = sb.tile([1, 2], f32)
            vnw_t = sb.tile([1, 16], f32)
            vnb_t = sb.tile([1, 8], f32)
            nc.scalar.dma_start(out=vnx_t[:], in_=vnx[:])
            nc.scalar.dma_start(out=vnw_t[:], in_=vnw[:])
            nc.scalar.dma_start(out=vnb_t[:], in_=vnb[:])
            vg_p = sb.tile([1, 8, 2], f32)
            nc.vector.tensor_mul(
                out=vg_p[:],
                in0=vnx_t[:][:, None, :].to_broadcast([1, 8, 2]),
                in1=vnw_t[:].rearrange("p (k t) -> p k t", t=2),
            )
            vg_r = sb.tile([1, 8], f32)
            nc.vector.tensor_reduce(out=vg_r[:][:, :, None], in_=vg_p[:], axis=X, op=Alu.add)
            vg = sb.tile([1, 8], f32)
            nc.vector.tensor_add(out=vg[:], in0=vg_r[:], in1=vnb_t[:])
            nc.scalar.dma_start(out=vg_stage[:], in_=vg[:])
            vgb = sb.tile([128, 8], f32)
            nc.gpsimd.dma_start(out=vgb[:], in_=vg_stage[:].to_broadcast([128, 8]))

            # virt half of the fully table: rows 0..511 = vg tiled 8x
            vt_t = sb.tile([128, 4, 64], f32)
            nc.vector.tensor_copy(
                out=vt_t[:].rearrange("p a (b c) -> p a b c", b=8),
                in_=vgb[:][:, None, None, :].to_broadcast([128, 4, 8, 8]),
            )
            nc.scalar.dma_start(out=f_tab[0:512, :].rearrange("(p r) e -> p (r e)", p=128), in_=vt_t[:])

            # ---- fully vector (f32r), split so stage A/B deps stay separate ----
            if vt > 0:
                fullyA = sb.tile([128, vt], f32r)
                oh8_t = sb.tile([128, vt, 8], f32)
                nc.scalar.dma_start(out=oh8_t[:], in_=oh8[:])
                fa_p = sb.tile([128, vt, 8], f32)
                nc.vector.tensor_mul(
                    out=fa_p[:], in0=oh8_t[:],
                    in1=vgb[:][:, None, :].to_broadcast([128, vt, 8]),
                )
                fa = sb.tile([128, vt], f32)
                nc.vector.tensor_reduce(out=fa[:][:, :, None], in_=fa_p[:], axis=X, op=Alu.add)
                fa_th = sb.tile([128, vt], f32)
                nc.scalar.activation(out=fa_th[:], in_=fa[:], func=Act.Tanh)
                nc.gpsimd.dma_start(out=fullyA[:], in_=fa_th[:])
                fA32 = fa_th
            if ct > 0:
                fullyB = sb.tile([128, ct], f32r)

            def lhs_col(t):
                if t < vt:
                    return fullyA[:, t:t + 1]
                return fullyB[:, t - vt:t - vt + 1]

            # ---- GEMV: stream pol_w; 8 psum chunk accumulators ----
            accs = [ps.tile([1, 512], f32, space="PSUM", tag=f"acc{j}", name=f"acc{j}") for j in range(8)]

            def acc_row(j):
                return accs[j][:]

            wts = []
            for t in range(KT):
                wt_t = stream.tile([128, N], f32r, tag="wt", name=f"wt{t}")
                nc.sync.dma_start(out=wt_t[:], in_=pol_w[t * 128:(t + 1) * 128, :])
                wts.append(wt_t)
                if t < vt:
                    for j in range(8):
                        nc.tensor.matmul(
                            out=acc_row(j), lhsT=lhs_col(t),
                            rhs=wt_t[:, j * 512:(j + 1) * 512],
                            start=(t == 0), stop=(t == KT - 1),
                        )

            def lhat(src_tab, pref_tab, F, cmp_buf, ar_buf, qbase):
                """gather src values (dst-sorted), prefix-scan, boundary-gather
                the prefix table, subtract -> per-node partial aggregates; AR."""
                gv = sb.tile([128, EPC // 128, EL], f32, tag="gath")
                for ci in range(EPC // GCH):
                    nc.gpsimd.dma_gather(
                        out_ap=gv[:, ci * (GCH // 128):(ci + 1) * (GCH // 128), :],
                        in_ap=src_tab[:], idxs_ap=srcg_t[:, ci * (GCH // 16):(ci + 1) * (GCH // 16)],
                        num_idxs=GCH, num_idxs_reg=GCH, elem_size=EL,
                        queue_num=0, single_packet=False,
                    )
                # per-partition inclusive scans along the 64 edges, one per feature
                pref = sb.tile([128, EPC // 128, EL], f32, tag="pref")
                for f in range(F):
                    nc.vector.tensor_tensor_scan(
                        out=pref[:, :, f], data0=gv[:, :, f], data1=gv[:, :, f],
                        initial=0.0, op0=Alu.add, op1=Alu.bypass,
                    )
                # cross-partition offsets: round-trip per-partition totals to one
                # row, scan along free, shift-load back (exclusive prefix)
                tot_d = dram.tile([128, F], f32, name=f"tot_d{qbase}_{F}")
                osc_d = dram.tile([128 * F], f32, name=f"osc_d{qbase}_{F}")
                nc.scalar.dma_start(out=tot_d[:], in_=pref[:, EPC // 128 - 1, 0:F])
                totrow = sb.tile([1, 128, F], f32, tag="totrow")
                nc.scalar.dma_start(out=totrow[:], in_=tot_d[:][None, :, :])
                for f in range(F):
                    nc.vector.tensor_tensor_scan(
                        out=totrow[:, :, f], data0=totrow[:, :, f], data1=totrow[:, :, f],
                        initial=0.0, op0=Alu.add, op1=Alu.bypass,
                    )
                nc.scalar.dma_start(out=osc_d[:][None, :], in_=totrow[:])
                offs_sb = sb.tile([128, 1, F], f32, tag="offs_sb")
                nc.vector.memset(offs_sb[:], 0.0)
                nc.scalar.dma_start(out=offs_sb[1:128, :, :], in_=osc_d[0:127 * F].rearrange("(p f) -> p f", p=127)[:, None, :])
                nc.vector.tensor_add(
                    out=pref[:, :, 0:F], in0=pref[:, :, 0:F],
                    in1=offs_sb[:].to_broadcast([128, EPC // 128, F]),
                )
                nc.scalar.dma_start(
                    out=pref_tab[1:EPC + 1, 0:F].rearrange("(q c) f -> q c f", q=128),
                    in_=pref[:, :, 0:F],
                )
                ghi = sb.tile([128, 32, EL], f32, tag="ghi")
                glo = sb.tile([128, 32, EL], f32, tag="glo")
                for ci in range(N // GCH):
                    s = slice(ci * (GCH // 16), (ci + 1) * (GCH // 16))
                    o = slice(ci * (GCH // 128), (ci + 1) * (GCH // 128))
                    nc.gpsimd.dma_gather(
                        out_ap=ghi[:, o, :], in_ap=pref_tab[:], idxs_ap=hi_t[:, s],
                        num_idxs=GCH, num_idxs_reg=GCH, elem_size=EL,
                        queue_num=0, single_packet=False,
                    )
                    nc.gpsimd.dma_gather(
                        out_ap=glo[:, o, :], in_ap=pref_tab[:], idxs_ap=lo_t[:, s],
                        num_idxs=GCH, num_idxs_reg=GCH, elem_size=EL,
                        queue_num=0, single_packet=False,
                    )
                agg = sb.tile([128, 32, F], f32, tag="agg")
                nc.vector.tensor_sub(out=agg[:], in0=ghi[:, :, 0:F], in1=glo[:, :, 0:F])
                nc.scalar.dma_start(out=cmp_buf[:].rearrange("(p c) f -> p c f", p=128), in_=agg[:])
                nc.gpsimd.collective_compute(
                    "AllReduce", Alu.add, replica_groups=RG,
                    ins=[cmp_buf[:].opt()], outs=[ar_buf[:].opt()],
                )
                res = sb.tile([128, 32, F], f32, tag="aggar")
                nc.scalar.dma_start(out=res[:], in_=ar_buf[:].rearrange("(p c) f -> p c f", p=128))
                return res

            # ---- lhat1 ----
            xd_t = sb.tile([128, 32, EL], f32)
            nc.vector.memset(xd_t[:], 0.0)
            nc.vector.tensor_mul(
                out=xd_t[:, :, 0:4], in0=x_t[:],
                in1=dinv[:][:, :, None].to_broadcast([128, 32, 4]),
            )
            nc.scalar.dma_start(out=xd_tab[:].rearrange("(p r) e -> p (r e)", p=128), in_=xd_t[:])
            yc = lhat(xd_tab, p1_tab, 4, y_cmp, y_ar, 1)

            # ---- z = x @ W1 + 2 * y @ W2 with y = -dinv * y_agg ----
            xy = sb.tile([128, 32, 8], f32)
            nc.vector.tensor_copy(out=xy[:, :, 0:4], in_=x_t[:])
            t1 = sb.tile([128, 32, 4], f32)
            nc.vector.tensor_mul(out=t1[:], in0=yc[:], in1=dinv[:][:, :, None].to_broadcast([128, 32, 4]))
            nc.vector.tensor_scalar_mul(out=xy[:, :, 4:8], in0=t1[:], scalar1=-1.0)
            z_p = sb.tile([128, 32, 8, 8], f32)
            nc.vector.tensor_mul(
                out=z_p[:],
                in0=xy[:][:, :, None, :].to_broadcast([128, 32, 8, 8]),
                in1=wcat_t[:][:, None, :, :].to_broadcast([128, 32, 8, 8]),
            )
            z_t = sb.tile([128, 32, 8], f32)
            nc.vector.tensor_reduce(out=z_t[:][:, :, :, None], in_=z_p[:], axis=X, op=Alu.add)
            zd_t = sb.tile([128, 32, EL], f32)
            nc.vector.memset(zd_t[:], 0.0)
            nc.vector.tensor_mul(
                out=zd_t[:, :, 0:8], in0=z_t[:],
                in1=dinv[:][:, :, None].to_broadcast([128, 32, 8]),
            )
            nc.scalar.dma_start(out=zd_tab[:].rearrange("(p r) e -> p (r e)", p=128), in_=zd_t[:])

            # ---- lhat2 ----
            lzc = lhat(zd_tab, p2_tab, 8, lz_cmp, lz_ar, 1)

            # ---- conv = relu(xw02 - dinv*lz_agg + cheb_b) ----
            lzd = sb.tile([128, 32, 8], f32)
            nc.vector.tensor_mul(out=lzd[:], in0=lzc[:], in1=dinv[:][:, :, None].to_broadcast([128, 32, 8]))
            conv0 = sb.tile([128, 32, 8], f32)
            nc.vector.tensor_sub(out=conv0[:], in0=xw02[:], in1=lzd[:])
            conv1 = sb.tile([128, 32, 8], f32)
            nc.vector.tensor_add(out=conv1[:], in0=conv0[:], in1=chb_t[:][:, None, :].to_broadcast([128, 32, 8]))
            conv = sb.tile([128, 32, 8], f32)
            nc.vector.tensor_relu(out=conv[:], in_=conv1[:])
            nc.scalar.dma_start(out=f_tab[512:1024, :].rearrange("(p r) e -> p (r e)", p=128), in_=conv[:])

            # ---- stage B fully entries: gather table rows by perm group, select offset ----
            if ct > 0:
                gf = sb.tile([128, ct, EL], f32, tag="gath")
                total = ct * 128
                nb = total // GCH
                rem = total - nb * GCH
                for ci in range(nb):
                    nc.gpsimd.dma_gather(
                        out_ap=gf[:, ci * (GCH // 128):(ci + 1) * (GCH // 128), :],
                        in_ap=f_tab[:], idxs_ap=grp_t[:, ci * (GCH // 16):(ci + 1) * (GCH // 16)],
                        num_idxs=GCH, num_idxs_reg=GCH, elem_size=EL,
                        queue_num=0, single_packet=False,
                    )
                if rem:
                    nc.gpsimd.dma_gather(
                        out_ap=gf[:, nb * (GCH // 128):, :],
                        in_ap=f_tab[:], idxs_ap=grp_t[:, nb * (GCH // 16):],
                        num_idxs=rem, num_idxs_reg=rem, elem_size=EL,
                        queue_num=0, single_packet=False,
                    )
                fb_p = sb.tile([128, ct, EL], f32, tag="pref")
                nc.vector.tensor_mul(out=fb_p[:], in0=gf[:], in1=oh64_t[:])
                fb = sb.tile([128, ct], f32)
                nc.vector.tensor_reduce(out=fb[:][:, :, None], in_=fb_p[:], axis=X, op=Alu.add)
                fb_th = sb.tile([128, ct], f32)
                nc.scalar.activation(out=fb_th[:], in_=fb[:], func=Act.Tanh)
                nc.gpsimd.dma_start(out=fullyB[:], in_=fb_th[:])
                fB32 = fb_th
            for t in range(vt, KT):
                for j in range(8):
                    nc.tensor.matmul(
                        out=acc_row(j), lhsT=lhs_col(t),
                        rhs=wts[t][:, j * 512:(j + 1) * 512],
                        start=(t == 0), stop=(t == KT - 1),
                    )

            # ---- copy logits out of PSUM, assemble AR input ----
            lsb = sb.tile([1, 4096], f32)
            for j in range(8):
                nc.vector.tensor_copy(out=lsb[:, j * 512:(j + 1) * 512], in_=acc_row(j))
            nc.scalar.dma_start(out=lg_in[0:1, 0:4096], in_=lsb[:])

            # ---- value partial: 64 [128,1]x[128,1] matmuls ----
            valw_t = sb.tile([128, KT], f32)
            nc.scalar.dma_start(out=valw_t[:], in_=val_w[:])
            vacc = ps.tile([1, 1], f32, space="PSUM", tag="acc0")
            for t in range(KT):
                f32col = fA32[:, t:t + 1] if t < vt else fB32[:, t - vt:t - vt + 1]
                nc.tensor.matmul(
                    out=vacc[:], lhsT=valw_t[:, t:t + 1], rhs=f32col,
                    start=(t == 0), stop=(t == KT - 1),
                )
            vsb = sb.tile([1, 1], f32)
            nc.vector.tensor_copy(out=vsb[:], in_=vacc[:])
            nc.scalar.dma_start(out=lg_in[:, 4096:4097], in_=vsb[:])

            nc.gpsimd.collective_compute(
                "AllReduce", Alu.add, replica_groups=RG,
                ins=[lg_in[:].opt()], outs=[lg_out[:].opt()],
            )

            # ---- softmax over 4096 (replicated on every core) ----
            lg_t = sb.tile([128, 32], f32)
            nc.scalar.dma_start(out=lg_t[:], in_=lg_out[0, 0:4096].rearrange("(p c) -> p c", p=128))
            pb_t = sb.tile([128, 32], f32)
            nc.scalar.dma_start(out=pb_t[:], in_=pol_b[:].rearrange("(p c) -> p c", p=128))
            lgb = sb.tile([128, 32], f32)
            nc.vector.tensor_add(out=lgb[:], in0=lg_t[:], in1=pb_t[:])
            rmax = sb.tile([128, 1], f32)
            nc.vector.tensor_reduce(out=rmax[:], in_=lgb[:], axis=X, op=Alu.max)
            ident = sb.tile([128, 128], f32)
            make_identity(nc, ident[:])
            rmax_tp = ps.tile([128, 128], f32, space="PSUM", tag="acc1")
            nc.tensor.transpose(out=rmax_tp[:], in_=rmax[:].to_broadcast([128, 128]), identity=ident[:])
            gmax = sb.tile([1, 1], f32)
            nc.vector.tensor_reduce(out=gmax[:], in_=rmax_tp[:1, :], axis=X, op=Alu.max)
            negones = sb.tile([1, 128], f32)
            nc.vector.memset(negones[:], -1.0)
            ngmax_b = ps.tile([128, 1], f32, space="PSUM", tag="acc2")
            nc.tensor.matmul(out=ngmax_b[:], lhsT=negones[:], rhs=gmax[:], start=True, stop=True)
            ngmax_sb = sb.tile([128, 1], f32)
            nc.vector.tensor_copy(out=ngmax_sb[:], in_=ngmax_b[:])
            ex = sb.tile([128, 32], f32)
            rsum = sb.tile([128, 1], f32)
            nc.scalar.activation(out=ex[:], in_=lgb[:], func=Act.Exp, bias=ngmax_sb[:], accum_out=rsum[:])
            onescol = sb.tile([128, 1], f32)
            nc.vector.memset(onescol[:], 1.0)
            gsum_p = ps.tile([1, 1], f32, space="PSUM", tag="acc3")
            nc.tensor.matmul(out=gsum_p[:], lhsT=rsum[:], rhs=onescol[:], start=True, stop=True)
            gsum = sb.tile([1, 1], f32)
            nc.vector.tensor_copy(out=gsum[:], in_=gsum_p[:])
            rinv = sb.tile([1, 1], f32)
            nc.vector.reciprocal(out=rinv[:], in_=gsum[:])
            onesrow = sb.tile([1, 128], f32)
            nc.vector.memset(onesrow[:], 1.0)
            rinv_b = ps.tile([128, 1], f32, space="PSUM", tag="acc4")
            nc.tensor.matmul(out=rinv_b[:], lhsT=onesrow[:], rhs=rinv[:], start=True, stop=True)
            rinv_sb = sb.tile([128, 1], f32)
            nc.vector.tensor_copy(out=rinv_sb[:], in_=rinv_b[:])
            pr = sb.tile([128, 32], f32)
            nc.vector.tensor_scalar_mul(out=pr[:], in0=ex[:], scalar1=rinv_sb[:])
            nc.scalar.dma_start(out=probs_o[:].rearrange("(p c) -> p c", p=128), in_=pr[:])

            # ---- value = AR(val_partial) + val_b ----
            vb_t = sb.tile([1, 1], f32)
            nc.scalar.dma_start(out=vb_t[:], in_=val_b[:, None])
            var_t = sb.tile([1, 1], f32)
            nc.scalar.dma_start(out=var_t[:], in_=lg_out[:, 4096:4097])
            vfin = sb.tile([1, 1], f32)
            nc.vector.tensor_add(out=vfin[:], in0=var_t[:], in1=vb_t[:])
            nc.scalar.dma_start(out=value_o[:, None], in_=vfin[:])

    nc.finalize()
    return nc


def _prepare(inputs):
    """Host-side sharding: slice pol_W/val_W rows, CSR-sort per-core edges by
    dst, build gather index layouts, sort per-core k-slots virt-first, build
    perm one-hots."""
    subs_x = np.ascontiguousarray(np.asarray(inputs["subs_x"], np.float32))
    vnr_x = np.asarray(inputs["vnr_x"], np.float32)
    cheb_W = np.asarray(inputs["cheb_W"], np.float32)
    cheb_b = np.asarray(inputs["cheb_b"], np.float32)
    vnr_W = np.asarray(inputs["vnr_W"], np.float32)
    vnr_b = np.asarray(inputs["vnr_b"], np.float32)
    pol_W = np.asarray(inputs["pol_W"], np.float32)
    pol_b = np.asarray(inputs["pol_b"], np.float32)
    val_W = np.asarray(inputs["val_W"], np.float32)
    val_b = np.asarray(inputs["val_b"], np.float32)
    edge_index = np.asarray(inputs["edge_index"])
    perm = np.asarray(inputs["perm"], np.int64)
    j = int(np.asarray(inputs["j"]))

    src = edge_index[0].astype(np.int64)
    dst = edge_index[1].astype(np.int64)

    # global source-degree CSR row pointers (for the sym-norm degree)
    deg = np.bincount(src, minlength=N)
    rp_src = np.concatenate([[0], np.cumsum(deg)]).astype(np.float32)

    # per-core k-slot ordering: virt entries (perm < 8N) first
    orders, vcounts = [], []
    for c in range(NCORES):
        kk = np.arange(c * KPC, (c + 1) * KPC)
        pv = perm[kk]
        is_virt = pv < 8 * N
        orders.append(np.concatenate([kk[is_virt], kk[~is_virt]]))
        vcounts.append(int(is_virt.sum()))
    if min(vcounts) == KPC:
        vt = KT
    else:
        vt = min(v // 128 for v in vcounts)
        vt = max(0, min(vt & ~1, KT - 2))

    w0t = cheb_W[0].T.reshape(1, 32).copy()
    w1t = cheb_W[1].T.reshape(1, 32).copy()
    w2t = cheb_W[2].T.reshape(1, 32).copy()
    ct = KT - vt

    # boundary-gather index layout: node n -> gather slot i=(n%32)*128 + n//32
    nn = np.arange(N)
    node_slot_n = (nn % 32) * 128 + nn // 32  # value = slot of node n
    inv_node = np.empty(N, np.int64)
    inv_node[node_slot_n] = nn  # inv_node[i] = node whose value lands at slot i

    in_maps = []
    for c in range(NCORES):
        order = orders[c]
        pv = perm[order]
        e0, e1 = c * EPC, (c + 1) * EPC
        # CSR sort this core's edges by dst (stable)
        sidx = np.argsort(dst[e0:e1], kind="stable")
        src_sorted = src[e0:e1][sidx]
        dst_sorted = dst[e0:e1][sidx]
        cnt = np.bincount(dst_sorted, minlength=N)
        rp = np.concatenate([[0], np.cumsum(cnt)])  # [N+1], values in [0, EPC]
        # gather idx array: slot i holds src of sorted-edge (i%128)*64 + i//128
        ii = np.arange(EPC)
        srcg = src_sorted[(ii % 128) * 64 + ii // 128].astype(np.int16)
        hi = (rp[inv_node + 1]).astype(np.int16)
        lo = (rp[inv_node]).astype(np.int16)
        m = {
            "pol_w": np.ascontiguousarray(pol_W[order]),
            "val_w": np.ascontiguousarray(val_W[order, 0].reshape(KT, 128).T),
            "pol_b": pol_b,
            "val_b": val_b,
            "x_in": subs_x,
            "w0t": w0t, "w1t": w1t, "w2t": w2t,
            "chb": cheb_b.reshape(1, 8).copy(),
            "vnx": vnr_x[j].reshape(1, 2).copy(),
            "vnw": vnr_W.T.reshape(1, 16).copy(),
            "vnb": vnr_b.reshape(1, 8).copy(),
            "rp_src": rp_src,
            "srcg_i": _wrap16(srcg),
            "hi_i": _wrap16(hi),
            "lo_i": _wrap16(lo),
        }
        if vt > 0:
            pa = pv[:vt * 128]
            oh = np.zeros((128, vt, 8), np.float32)
            ia = np.arange(vt * 128)
            oh[ia % 128, ia // 128, (pa % 8).astype(np.int64)] = 1.0
            m["oh8"] = oh
        if ct > 0:
            pb = pv[vt * 128:]
            m["grp_i"] = _wrap16((pb // 64).astype(np.int16))
            oh = np.zeros((128, ct, 64), np.float32)
            ib = np.arange(ct * 128)
            oh[ib % 128, ib // 128, (pb % 64).astype(np.int64)] = 1.0
            m["oh64"] = oh
        in_maps.append(m)
    return vt, in_maps


def kernel(**inputs):
    from concourse.bass_utils import run_bass_kernel_spmd

    vt, in_maps = _prepare(inputs)
    nc = _BUILD_CACHE.get(vt)
    if nc is None:
        nc = _build(vt)
        _BUILD_CACHE[vt] = nc
    res = run_bass_kernel_spmd(nc, in_maps, core_ids=list(range(NCORES)))
    r0 = res.results[0]
    return np.asarray(r0["probs"], np.float32), np.asarray(r0["value"], np.float32)
